# revision 8
# baseline (speedup 1.0000x reference)
"""Trainium2 Bass kernel for nn_AttentionBlock (GroupNorm + single-head self-attention + proj + residual).

Input  x [4, 512, 64, 64] f32.  8 NeuronCores: core i handles batch b=i//2,
query-half h=i%2 (2048 of the 4096 spatial positions).  Each core computes
GroupNorm + full K/V for its batch element, Q only for its half, flash-style
attention over all 4096 keys, the output projection and residual for its half.
No collectives: host shards inputs / gathers outputs.

Layouts (per core), n=4096, nq=2048, c=512:
  hf  = GN(x)      [c, n]    (4 SBUF tiles [128, 4096], in-place over x)
  k   = Wk hf + bk [c, n]    (resident SBUF, 4 tiles [128, 4096])
  q   = Wq hf + bq [c, nq]   (spilled to DRAM, streamed back per q-chunk)
  v^T = (Wv hf + bv)^T [n, c] (spilled to DRAM, streamed back per (qc, j))
  S^T chunk [j:128, q:512] = k_chunk^T q_chunk   (PSUM)
  E = exp(S^T * 1/sqrt(c))  -> O[co,q] += v^T_j[:,co]^T E ;  denom[1,q] += 1^T E
  y^T [q:128, c:512] = (O chunk)^T Wp^T ; y = y^T * (1/denom) + (x^T + b_proj)
"""

import os
import numpy as np

B, C, HH, WW = 4, 512, 64, 64
N = HH * WW            # 4096
NQ = N // 2            # 2048 queries per core
NCORES = 8
CT = C // 128          # 4 channel tiles
PT = N // 512          # 8 spatial chunks of 512
QT = NQ // 512         # 4 query chunks of 512
JT = N // 128          # 32 key chunks of 128
GSIZE = 16             # channels per group
EPS = 1e-5
SCALE = 1.0 / float(np.sqrt(C))

MM_DT_NAME = os.environ.get("KERNEL_MM_DT", "float32")

_PROG = None
_PROG_DT = None


def _build_program(mm_dt_name):
    import concourse.bacc as bacc
    import concourse.tile as tile
    from concourse import mybir
    from contextlib import ExitStack

    F32 = mybir.dt.float32
    MM = getattr(mybir.dt, mm_dt_name)

    nc = bacc.Bacc("TRN2", target_bir_lowering=False, debug=False,
                   num_devices=NCORES)

    def din(name, shape):
        return nc.dram_tensor(name, shape, F32, kind="ExternalInput").ap()

    x_cn = din("x_cn", [C, N])          # x for this batch, query-half first
    xb_t = din("xb_t", [NQ, C])         # x^T residual slice + b_proj
    w_qT = din("w_qT", [C, C])
    w_kT = din("w_kT", [C, C])
    w_vT = din("w_vT", [C, C])
    w_pT = din("w_pT", [C, C])
    b_q = din("b_q", [C, 1])
    b_k = din("b_k", [C, 1])
    b_v = din("b_v", [1, C])
    gam = din("gam", [C, 1])
    bet = din("bet", [C, 1])
    gmat = din("gmat", [128, 8])        # group membership (p//16 == u)
    gmat_t = din("gmat_t", [8, 128])
    y_t = nc.dram_tensor("y_t", [NQ, C], F32, kind="ExternalOutput").ap()

    AF = mybir.ActivationFunctionType
    OP = mybir.AluOpType

    with tile.TileContext(nc) as tc, ExitStack() as ctx:
        persist = ctx.enter_context(tc.tile_pool(name="persist", bufs=1))
        kpool = ctx.enter_context(tc.tile_pool(name="kpool", bufs=1))
        dram = ctx.enter_context(tc.tile_pool(name="dram", bufs=1, space="DRAM"))

        # ---- persistent small constants ----
        gma = persist.tile([128, 8], F32)
        nc.sync.dma_start(out=gma, in_=gmat)
        gmt = persist.tile([8, 128], F32)
        nc.sync.dma_start(out=gmt, in_=gmat_t)
        one1 = persist.tile([1, 1], F32)
        nc.vector.memset(one1, 1.0)
        ones_row = persist.tile([1, 128], MM)
        ones_col = persist.tile([128, 1], MM)
        if MM is F32:
            nc.vector.memset(ones_row, 1.0)
            nc.vector.memset(ones_col, 1.0)
        else:
            ones_st_r = persist.tile([1, 128], F32)
            nc.vector.memset(ones_st_r, 1.0)
            nc.gpsimd.tensor_copy(ones_row, ones_st_r)
            ones_st_c = persist.tile([128, 1], F32)
            nc.vector.memset(ones_st_c, 1.0)
            nc.gpsimd.tensor_copy(ones_col, ones_st_c)
        eps8 = persist.tile([8, 1], F32)
        nc.vector.memset(eps8, EPS)
        gcol = []
        bcol = []
        bqcol = []
        bkcol = []
        for t in range(CT):
            g_t = persist.tile([128, 1], F32, name=f"g_{t}")
            nc.sync.dma_start(out=g_t, in_=gam[t * 128:(t + 1) * 128, :])
            gcol.append(g_t)
            be_t = persist.tile([128, 1], F32, name=f"be_{t}")
            nc.sync.dma_start(out=be_t, in_=bet[t * 128:(t + 1) * 128, :])
            bcol.append(be_t)
            bq_t = persist.tile([128, 1], F32, name=f"bq_{t}")
            nc.sync.dma_start(out=bq_t, in_=b_q[t * 128:(t + 1) * 128, :])
            bqcol.append(bq_t)
            bk_t = persist.tile([128, 1], F32, name=f"bk_{t}")
            nc.sync.dma_start(out=bk_t, in_=b_k[t * 128:(t + 1) * 128, :])
            bkcol.append(bk_t)
        # b_v row, rounded to MM dtype
        bvr = persist.tile([1, C], MM)
        if MM is F32:
            nc.sync.dma_start(out=bvr, in_=b_v)
        else:
            bvr_st = persist.tile([1, C], F32)
            nc.sync.dma_start(out=bvr_st, in_=b_v)
            nc.gpsimd.tensor_copy(bvr, bvr_st)
        # w_proj^T tiles (persist: used at the end of every q-chunk)
        wp = []
        for t in range(CT):
            wp_t = persist.tile([128, C], MM, name=f"wp_{t}")
            if MM is F32:
                nc.sync.dma_start(out=wp_t, in_=w_pT[t * 128:(t + 1) * 128, :])
            else:
                wp_st = persist.tile([128, C], F32, name=f"wpst_{t}")
                nc.sync.dma_start(out=wp_st, in_=w_pT[t * 128:(t + 1) * 128, :])
                nc.gpsimd.tensor_copy(wp_t, wp_st)
            wp.append(wp_t)

        # k stays resident for the whole kernel
        k_tiles = [kpool.tile([128, N], MM, name=f"k_{t}", tag=f"k{t}")
                   for t in range(CT)]
        # spill targets
        vt_dram = dram.tile([JT, 128, C], MM)     # v^T as 32 j-tiles [128, 512]
        q_dram = dram.tile([CT, 128, NQ], MM)     # q in [c, nq] layout

        with tc.tile_pool(name="hfpool", bufs=1) as hfpool:
            hf = [hfpool.tile([128, N], MM, name=f"hf_{t}", tag=f"hf{t}")
                  for t in range(CT)]

            # ---------------- GroupNorm ----------------
            # per c-tile: load x, per-channel stats, group-aggregate (groups
            # never span c-tiles), expand, apply -> hf (rounded to MM dtype)
            with tc.tile_pool(name="xpool", bufs=2) as xpool, \
                 tc.tile_pool(name="gnsb", bufs=2) as gnsb, \
                 tc.tile_pool(name="gnps", bufs=2, space="PSUM") as gnps:
                for t in range(CT):
                    x_t = xpool.tile([128, N], F32, tag="x")
                    nc.sync.dma_start(out=x_t,
                                      in_=x_cn[t * 128:(t + 1) * 128, :])
                    stats = gnsb.tile([128, PT, 6], F32, tag="stats")
                    for s in range(PT):
                        nc.vector.bn_stats(out=stats[:, s, :],
                                           in_=x_t[:, s * 512:(s + 1) * 512])
                    mv = gnsb.tile([128, 2], F32, tag="mv")
                    nc.vector.bn_aggr(out=mv, in_=stats)
                    # st2 = [mean, E[x^2]] per channel
                    st2 = gnsb.tile([128, 2], F32, tag="st2")
                    nc.vector.tensor_copy(st2[:, 0:1], mv[:, 0:1])
                    msq = gnsb.tile([128, 1], F32, tag="msq")
                    nc.vector.tensor_mul(msq, mv[:, 0:1], mv[:, 0:1])
                    nc.vector.tensor_add(st2[:, 1:2], mv[:, 1:2], msq)
                    # group-aggregate: [8, 2] sums over the 16 channels of each group
                    gps = gnps.tile([8, 2], F32, tag="gps")
                    nc.tensor.matmul(gps, gma, st2, start=True, stop=True)
                    grp = gnsb.tile([8, 2], F32, tag="grp")
                    nc.scalar.mul(out=grp, in_=gps, mul=1.0 / GSIZE)
                    gm2 = gnsb.tile([8, 1], F32, tag="gm2")
                    nc.vector.tensor_mul(gm2, grp[:, 0:1], grp[:, 0:1])
                    var = gnsb.tile([8, 1], F32, tag="var")
                    nc.vector.tensor_sub(var, grp[:, 1:2], gm2)
                    std = gnsb.tile([8, 1], F32, tag="std")
                    nc.scalar.activation(out=std, in_=var, func=AF.Sqrt,
                                         bias=eps8, scale=1.0)
                    gout = gnsb.tile([8, 2], F32, tag="gout")
                    nc.vector.tensor_copy(gout[:, 0:1], grp[:, 0:1])
                    nc.vector.reciprocal(out=gout[:, 1:2], in_=std)
                    # expand group stats back to per-channel [128, 2]
                    eps_ps = gnps.tile([128, 2], F32, tag="eps_ps")
                    nc.tensor.matmul(eps_ps, gmt, gout, start=True, stop=True)
                    pg = gnsb.tile([128, 2], F32, tag="pg")
                    nc.scalar.copy(out=pg, in_=eps_ps)
                    # per-channel scale = gamma*rstd ; bias = beta - mean*scale
                    sc = gnsb.tile([128, 1], F32, tag="sc")
                    nc.vector.tensor_mul(sc, gcol[t], pg[:, 1:2])
                    bc = gnsb.tile([128, 1], F32, tag="bc")
                    nc.vector.tensor_mul(bc, pg[:, 0:1], sc)
                    nc.vector.tensor_sub(bc, bcol[t], bc)
                    # apply: hf = x*sc + bc  (rounds to MM dtype)
                    nc.vector.tensor_scalar(out=hf[t], in0=x_t,
                                            scalar1=sc, scalar2=bc,
                                            op0=OP.mult, op1=OP.add)

            # ---------------- QKV ----------------
            with tc.tile_pool(name="wmat", bufs=1) as wmat, \
                 tc.tile_pool(name="qkvsb", bufs=3) as qkvsb, \
                 tc.tile_pool(name="qkvps", bufs=4, space="PSUM") as qkvps:

                def load_w(src, nm):
                    tiles = []
                    for t in range(CT):
                        w_t = wmat.tile([128, C], MM, name=f"{nm}{t}",
                                        tag=f"w{t}", bufs=2)
                        if MM is F32:
                            nc.sync.dma_start(out=w_t,
                                              in_=src[t * 128:(t + 1) * 128, :])
                        else:
                            w_st = wmat.tile([128, C], F32, name=f"{nm}st{t}",
                                             tag="wst", bufs=2)
                            nc.sync.dma_start(out=w_st,
                                              in_=src[t * 128:(t + 1) * 128, :])
                            nc.gpsimd.tensor_copy(w_t, w_st)
                        tiles.append(w_t)
                    return tiles

                # v^T = hf^T w_vT + b_v : 32 tiles [128p, 512c] -> DRAM
                wv = load_w(w_vT, "wv")
                for p in range(JT):
                    vt_ps = qkvps.tile([128, C], F32, tag="mm")
                    for c in range(CT):
                        nc.tensor.matmul(vt_ps,
                                         hf[c][:, p * 128:(p + 1) * 128],
                                         wv[c], start=(c == 0), stop=False)
                    nc.tensor.matmul(vt_ps, ones_row, bvr, start=False, stop=True)
                    vt_sb = qkvsb.tile([128, C], MM, tag="vt")
                    nc.scalar.copy(out=vt_sb, in_=vt_ps)
                    nc.sync.dma_start(out=vt_dram[p], in_=vt_sb)

                # k = w_kT^T hf + b_k : resident [c,n] tiles
                wk = load_w(w_kT, "wk")
                for o in range(CT):
                    for p in range(PT):
                        k_ps = qkvps.tile([128, 512], F32, tag="mm")
                        for c in range(CT):
                            nc.tensor.matmul(k_ps,
                                             wk[c][:, o * 128:(o + 1) * 128],
                                             hf[c][:, p * 512:(p + 1) * 512],
                                             start=(c == 0), stop=(c == CT - 1))
                        nc.vector.tensor_scalar_add(
                            out=k_tiles[o][:, p * 512:(p + 1) * 512],
                            in0=k_ps, scalar1=bkcol[o])

                # q = w_qT^T hf + b_q for first NQ columns -> DRAM
                wq = load_w(w_qT, "wq")
                for o in range(CT):
                    for p in range(QT):
                        q_ps = qkvps.tile([128, 512], F32, tag="mm")
                        for c in range(CT):
                            nc.tensor.matmul(q_ps,
                                             wq[c][:, o * 128:(o + 1) * 128],
                                             hf[c][:, p * 512:(p + 1) * 512],
                                             start=(c == 0), stop=(c == CT - 1))
                        q_sb = qkvsb.tile([128, 512], MM, tag="q")
                        nc.vector.tensor_scalar_add(out=q_sb, in0=q_ps,
                                                    scalar1=bqcol[o])
                        nc.sync.dma_start(
                            out=q_dram[o][:, p * 512:(p + 1) * 512], in_=q_sb)

        # ---------------- attention + proj (per 512-wide q-chunk) ----------------
        with tc.tile_pool(name="qa", bufs=2) as qapool, \
             tc.tile_pool(name="estream", bufs=3) as epool, \
             tc.tile_pool(name="vstream", bufs=4) as vpool, \
             tc.tile_pool(name="osb", bufs=2) as opool, \
             tc.tile_pool(name="ysb", bufs=2) as ypool, \
             tc.tile_pool(name="xbst", bufs=3) as xbpool, \
             tc.tile_pool(name="dsb", bufs=2) as dpool, \
             tc.tile_pool(name="psS", bufs=2, space="PSUM") as psS, \
             tc.tile_pool(name="psO", bufs=1, space="PSUM") as psO, \
             tc.tile_pool(name="psD", bufs=2, space="PSUM") as psD:

            for qc in range(QT):
                # stream this q-chunk back: qa[c] = q[c, qc*512 : +512]
                qa = []
                for c in range(CT):
                    qa_c = qapool.tile([128, 512], MM, name=f"qa_{c}",
                                       tag=f"qa{c}")
                    nc.sync.dma_start(out=qa_c,
                                      in_=q_dram[c][:, qc * 512:(qc + 1) * 512])
                    qa.append(qa_c)

                o_ps = [psO.tile([128, 512], F32, name=f"o_ps{co}",
                                 tag=f"o{co}") for co in range(CT)]
                d_ps = psD.tile([1, 512], F32, tag="d")

                def s_exp_v(j):
                    # S^T chunk + exp, and prefetch v^T tile for this j
                    vt_sb = vpool.tile([128, C], MM, tag="vt2")
                    nc.sync.dma_start(out=vt_sb, in_=vt_dram[j])
                    s_ps = psS.tile([128, 512], F32, tag="s")
                    for c in range(CT):
                        nc.tensor.matmul(s_ps,
                                         k_tiles[c][:, j * 128:(j + 1) * 128],
                                         qa[c], start=(c == 0), stop=(c == CT - 1))
                    e_sb = epool.tile([128, 512], MM, tag="e")
                    nc.scalar.activation(out=e_sb, in_=s_ps, func=AF.Exp,
                                         scale=SCALE)
                    return e_sb, vt_sb

                e_cur, v_cur = s_exp_v(0)
                for j in range(JT):
                    nxt = s_exp_v(j + 1) if j + 1 < JT else None
                    first, last = (j == 0), (j == JT - 1)
                    for co in range(CT):
                        nc.tensor.matmul(o_ps[co],
                                         v_cur[:, co * 128:(co + 1) * 128],
                                         e_cur, start=first, stop=last)
                    nc.tensor.matmul(d_ps, ones_col, e_cur,
                                     start=first, stop=last)
                    if nxt is not None:
                        e_cur, v_cur = nxt

                # denominators -> per-query reciprocal [128,1] per 128-row block
                d_sb = dpool.tile([1, 512], F32, tag="dsb")
                nc.scalar.copy(out=d_sb, in_=d_ps)
                rc = []
                for qs in range(4):
                    dt_ps = psS.tile([128, 1], F32, name=f"dt_ps{qs}", tag="s")
                    nc.tensor.transpose(dt_ps,
                                        d_sb[0:1, qs * 128:(qs + 1) * 128],
                                        one1)
                    rc_t = dpool.tile([128, 1], F32, name=f"rc_{qs}",
                                      tag=f"rc{qs}")
                    nc.vector.reciprocal(out=rc_t, in_=dt_ps)
                    rc.append(rc_t)

                # O -> SBUF (rounds to MM dtype)
                o_sb = []
                for co in range(CT):
                    o_t = opool.tile([128, 512], MM, name=f"o_sb{co}",
                                     tag=f"ob{co}")
                    nc.vector.tensor_copy(o_t, o_ps[co])
                    o_sb.append(o_t)

                # proj + 1/denom + residual, per 128-row output block
                for qs in range(4):
                    y_ps = psO.tile([128, C], F32, name=f"y_ps{qs}",
                                    tag=f"o{qs}")
                    for c in range(CT):
                        nc.tensor.matmul(y_ps,
                                         o_sb[c][:, qs * 128:(qs + 1) * 128],
                                         wp[c], start=(c == 0), stop=(c == CT - 1))
                    row0 = qc * 512 + qs * 128
                    xb_sb = xbpool.tile([128, C], F32, tag="xb")
                    nc.sync.dma_start(out=xb_sb, in_=xb_t[row0:row0 + 128, :])
                    y1 = ypool.tile([128, C], F32, tag="y1")
                    nc.vector.tensor_scalar_mul(out=y1, in0=y_ps, scalar1=rc[qs])
                    yo = ypool.tile([128, C], F32, tag="yo")
                    nc.vector.tensor_add(yo, y1, xb_sb)
                    nc.sync.dma_start(out=y_t[row0:row0 + 128, :], in_=yo)

    nc.compile()
    return nc


def _get_prog():
    global _PROG, _PROG_DT
    if _PROG is None or _PROG_DT != MM_DT_NAME:
        _PROG = _build_program(MM_DT_NAME)
        _PROG_DT = MM_DT_NAME
    return _PROG


def kernel(x, gamma, beta, w_qkv, b_qkv, w_proj, b_proj):
    from concourse.bass_utils import run_bass_kernel_spmd

    x = np.asarray(x, dtype=np.float32)
    gamma = np.asarray(gamma, dtype=np.float32)
    beta = np.asarray(beta, dtype=np.float32)
    w_qkv = np.asarray(w_qkv, dtype=np.float32)
    b_qkv = np.asarray(b_qkv, dtype=np.float32)
    w_proj = np.asarray(w_proj, dtype=np.float32)
    b_proj = np.asarray(b_proj, dtype=np.float32)

    shared = {
        "w_qT": np.ascontiguousarray(w_qkv[0:C].T),
        "w_kT": np.ascontiguousarray(w_qkv[C:2 * C].T),
        "w_vT": np.ascontiguousarray(w_qkv[2 * C:3 * C].T),
        "w_pT": np.ascontiguousarray(w_proj.T),
        "b_q": np.ascontiguousarray(b_qkv[0:C].reshape(C, 1)),
        "b_k": np.ascontiguousarray(b_qkv[C:2 * C].reshape(C, 1)),
        "b_v": np.ascontiguousarray(b_qkv[2 * C:3 * C].reshape(1, C)),
        "gam": np.ascontiguousarray(gamma.reshape(C, 1)),
        "bet": np.ascontiguousarray(beta.reshape(C, 1)),
        "gmat": (np.arange(128)[:, None] // GSIZE ==
                 np.arange(8)[None, :]).astype(np.float32),
        "gmat_t": np.ascontiguousarray(
            (np.arange(128)[:, None] // GSIZE ==
             np.arange(8)[None, :]).astype(np.float32).T),
    }

    in_maps = []
    for i in range(NCORES):
        b, h = i // 2, i % 2
        x2 = x[b].reshape(C, N)
        if h == 0:
            x_cn = np.ascontiguousarray(x2)
        else:
            x_cn = np.ascontiguousarray(
                np.concatenate([x2[:, NQ:], x2[:, :NQ]], axis=1))
        xb = np.ascontiguousarray(x2.T[h * NQ:(h + 1) * NQ] + b_proj[None, :])
        m = {"x_cn": x_cn, "xb_t": xb}
        m.update(shared)
        in_maps.append(m)

    nc = _get_prog()
    trace = os.environ.get("KERNEL_TRACE", "0") == "1"
    res = run_bass_kernel_spmd(nc, in_maps, list(range(NCORES)), trace=trace)
    if trace:
        kernel.last_exec_time_ns = res.exec_time_ns
        kernel.last_results = res

    out = np.empty((B, C, N), dtype=np.float32)
    for i in range(NCORES):
        b, h = i // 2, i % 2
        out[b][:, h * NQ:(h + 1) * NQ] = res.results[i]["y_t"].T
    return out.reshape(B, C, HH, WW)


# revision 10
# speedup vs baseline: 1.0011x; 1.0011x over previous
"""Trainium2 Bass kernel for nn_AttentionBlock (GroupNorm + single-head self-attention + proj + residual).

Input  x [4, 512, 64, 64] f32.  8 NeuronCores: core i handles batch b=i//2,
query-half h=i%2 (2048 of the 4096 spatial positions).  Each core computes
GroupNorm + full K/V for its batch element, Q only for its half, flash-style
attention over all 4096 keys, the output projection and residual for its half.
No collectives: host shards inputs / gathers outputs.

Layouts (per core), n=4096, nq=2048, c=512:
  hf  = GN(x)      [c, n]    (4 SBUF tiles [128, 4096], in-place over x)
  k   = Wk hf + bk [c, n]    (resident SBUF, 4 tiles [128, 4096])
  q   = Wq hf + bq [c, nq]   (spilled to DRAM, streamed back per q-chunk)
  v^T = (Wv hf + bv)^T [n, c] (spilled to DRAM, streamed back per (qc, j))
  S^T chunk [j:128, q:512] = k_chunk^T q_chunk   (PSUM)
  E = exp(S^T * 1/sqrt(c))  -> O[co,q] += v^T_j[:,co]^T E ;  denom[1,q] += 1^T E
  y^T [q:128, c:512] = (O chunk)^T Wp^T ; y = y^T * (1/denom) + (x^T + b_proj)
"""

import os
import numpy as np

B, C, HH, WW = 4, 512, 64, 64
N = HH * WW            # 4096
NQ = N // 2            # 2048 queries per core
NCORES = 8
CT = C // 128          # 4 channel tiles
PT = N // 512          # 8 spatial chunks of 512
QT = NQ // 512         # 4 query chunks of 512
JT = N // 128          # 32 key chunks of 128
GSIZE = 16             # channels per group
EPS = 1e-5
SCALE = 1.0 / float(np.sqrt(C))

MM_DT_NAME = os.environ.get("KERNEL_MM_DT", "float32")

_PROG = None
_PROG_DT = None


def _build_program(mm_dt_name):
    import concourse.bacc as bacc
    import concourse.tile as tile
    from concourse import mybir
    from contextlib import ExitStack

    F32 = mybir.dt.float32
    MM = getattr(mybir.dt, mm_dt_name)

    nc = bacc.Bacc("TRN2", target_bir_lowering=False, debug=False,
                   num_devices=NCORES)

    def din(name, shape, dt=None):
        return nc.dram_tensor(name, shape, dt or F32, kind="ExternalInput").ap()

    x_cn = din("x_cn", [C, N])          # x for this batch, query-half first
    xb_t = din("xb_t", [NQ, C])         # x^T residual slice + b_proj
    w_qT = din("w_qT", [C, C], MM)
    w_kT = din("w_kT", [C, C], MM)
    w_vT = din("w_vT", [C, C], MM)
    w_pT = din("w_pT", [C, C], MM)
    b_q = din("b_q", [C, 1])
    b_k = din("b_k", [C, 1])
    b_v = din("b_v", [1, C], MM)
    gam = din("gam", [C, 1])
    bet = din("bet", [C, 1])
    gmat = din("gmat", [128, 8])        # group membership (p//16 == u)
    gmat_t = din("gmat_t", [8, 128])
    y_t = nc.dram_tensor("y_t", [NQ, C], F32, kind="ExternalOutput").ap()

    AF = mybir.ActivationFunctionType
    OP = mybir.AluOpType

    with tile.TileContext(nc) as tc, ExitStack() as ctx:
        persist = ctx.enter_context(tc.tile_pool(name="persist", bufs=1))
        kpool = ctx.enter_context(tc.tile_pool(name="kpool", bufs=1))
        dram = ctx.enter_context(tc.tile_pool(name="dram", bufs=1, space="DRAM"))

        # ---- persistent small constants ----
        gma = persist.tile([128, 8], F32)
        nc.sync.dma_start(out=gma, in_=gmat)
        gmt = persist.tile([8, 128], F32)
        nc.sync.dma_start(out=gmt, in_=gmat_t)
        one1 = persist.tile([1, 1], F32)
        nc.vector.memset(one1, 1.0)
        ones_row = persist.tile([1, 128], MM)
        ones_col = persist.tile([128, 1], MM)
        if MM is F32:
            nc.vector.memset(ones_row, 1.0)
            nc.vector.memset(ones_col, 1.0)
        else:
            ones_st_r = persist.tile([1, 128], F32)
            nc.vector.memset(ones_st_r, 1.0)
            nc.vector.tensor_copy(ones_row, ones_st_r)
            ones_st_c = persist.tile([128, 1], F32)
            nc.vector.memset(ones_st_c, 1.0)
            nc.vector.tensor_copy(ones_col, ones_st_c)
        eps8 = persist.tile([8, 1], F32)
        nc.vector.memset(eps8, EPS)
        gcol = []
        bcol = []
        bqcol = []
        bkcol = []
        for t in range(CT):
            g_t = persist.tile([128, 1], F32, name=f"g_{t}")
            nc.sync.dma_start(out=g_t, in_=gam[t * 128:(t + 1) * 128, :])
            gcol.append(g_t)
            be_t = persist.tile([128, 1], F32, name=f"be_{t}")
            nc.sync.dma_start(out=be_t, in_=bet[t * 128:(t + 1) * 128, :])
            bcol.append(be_t)
            bq_t = persist.tile([128, 1], F32, name=f"bq_{t}")
            nc.sync.dma_start(out=bq_t, in_=b_q[t * 128:(t + 1) * 128, :])
            bqcol.append(bq_t)
            bk_t = persist.tile([128, 1], F32, name=f"bk_{t}")
            nc.sync.dma_start(out=bk_t, in_=b_k[t * 128:(t + 1) * 128, :])
            bkcol.append(bk_t)
        # b_v row (already MM dtype in DRAM)
        bvr = persist.tile([1, C], MM)
        nc.sync.dma_start(out=bvr, in_=b_v)
        # w_proj^T tiles (persist: used at the end of every q-chunk)
        wp = []
        for t in range(CT):
            wp_t = persist.tile([128, C], MM, name=f"wp_{t}")
            nc.sync.dma_start(out=wp_t, in_=w_pT[t * 128:(t + 1) * 128, :])
            wp.append(wp_t)

        # k stays resident for the whole kernel
        k_tiles = [kpool.tile([128, N], MM, name=f"k_{t}", tag=f"k{t}")
                   for t in range(CT)]
        # spill targets
        vt_dram = dram.tile([JT, 128, C], MM)     # v^T as 32 j-tiles [128, 512]
        q_dram = dram.tile([CT, 128, NQ], MM)     # q in [c, nq] layout

        with tc.tile_pool(name="hfpool", bufs=1) as hfpool:
            hf = [hfpool.tile([128, N], MM, name=f"hf_{t}", tag=f"hf{t}")
                  for t in range(CT)]

            # ---------------- GroupNorm ----------------
            # per c-tile: load x, per-channel stats, group-aggregate (groups
            # never span c-tiles), expand, apply -> hf (rounded to MM dtype)
            with tc.tile_pool(name="xpool", bufs=2) as xpool, \
                 tc.tile_pool(name="gnsb", bufs=2) as gnsb, \
                 tc.tile_pool(name="gnps", bufs=2, space="PSUM") as gnps:
                for t in range(CT):
                    x_t = xpool.tile([128, N], F32, tag="x")
                    nc.sync.dma_start(out=x_t,
                                      in_=x_cn[t * 128:(t + 1) * 128, :])
                    stats = gnsb.tile([128, PT, 6], F32, tag="stats")
                    for s in range(PT):
                        nc.vector.bn_stats(out=stats[:, s, :],
                                           in_=x_t[:, s * 512:(s + 1) * 512])
                    mv = gnsb.tile([128, 2], F32, tag="mv")
                    nc.vector.bn_aggr(out=mv, in_=stats)
                    # st2 = [mean, E[x^2]] per channel
                    st2 = gnsb.tile([128, 2], F32, tag="st2")
                    nc.vector.tensor_copy(st2[:, 0:1], mv[:, 0:1])
                    msq = gnsb.tile([128, 1], F32, tag="msq")
                    nc.vector.tensor_mul(msq, mv[:, 0:1], mv[:, 0:1])
                    nc.vector.tensor_add(st2[:, 1:2], mv[:, 1:2], msq)
                    # group-aggregate: [8, 2] sums over the 16 channels of each group
                    gps = gnps.tile([8, 2], F32, tag="gps")
                    nc.tensor.matmul(gps, gma, st2, start=True, stop=True)
                    grp = gnsb.tile([8, 2], F32, tag="grp")
                    nc.scalar.mul(out=grp, in_=gps, mul=1.0 / GSIZE)
                    gm2 = gnsb.tile([8, 1], F32, tag="gm2")
                    nc.vector.tensor_mul(gm2, grp[:, 0:1], grp[:, 0:1])
                    var = gnsb.tile([8, 1], F32, tag="var")
                    nc.vector.tensor_sub(var, grp[:, 1:2], gm2)
                    std = gnsb.tile([8, 1], F32, tag="std")
                    nc.scalar.activation(out=std, in_=var, func=AF.Sqrt,
                                         bias=eps8, scale=1.0)
                    gout = gnsb.tile([8, 2], F32, tag="gout")
                    nc.vector.tensor_copy(gout[:, 0:1], grp[:, 0:1])
                    nc.vector.reciprocal(out=gout[:, 1:2], in_=std)
                    # expand group stats back to per-channel [128, 2]
                    eps_ps = gnps.tile([128, 2], F32, tag="eps_ps")
                    nc.tensor.matmul(eps_ps, gmt, gout, start=True, stop=True)
                    pg = gnsb.tile([128, 2], F32, tag="pg")
                    nc.scalar.copy(out=pg, in_=eps_ps)
                    # per-channel scale = gamma*rstd ; bias = beta - mean*scale
                    sc = gnsb.tile([128, 1], F32, tag="sc")
                    nc.vector.tensor_mul(sc, gcol[t], pg[:, 1:2])
                    bc = gnsb.tile([128, 1], F32, tag="bc")
                    nc.vector.tensor_mul(bc, pg[:, 0:1], sc)
                    nc.vector.tensor_sub(bc, bcol[t], bc)
                    # apply: hf = x*sc + bc  (rounds to MM dtype)
                    nc.vector.tensor_scalar(out=hf[t], in0=x_t,
                                            scalar1=sc, scalar2=bc,
                                            op0=OP.mult, op1=OP.add)

            # ---------------- QKV ----------------
            with tc.tile_pool(name="wmat", bufs=1) as wmat, \
                 tc.tile_pool(name="qkvsb", bufs=3) as qkvsb, \
                 tc.tile_pool(name="qkvps", bufs=4, space="PSUM") as qkvps:

                def load_w(src, nm):
                    tiles = []
                    for t in range(CT):
                        w_t = wmat.tile([128, C], MM, name=f"{nm}{t}",
                                        tag=f"w{t}", bufs=2)
                        nc.sync.dma_start(out=w_t,
                                          in_=src[t * 128:(t + 1) * 128, :])
                        tiles.append(w_t)
                    return tiles

                # v^T = hf^T w_vT + b_v : 32 tiles [128p, 512c] -> DRAM
                wv = load_w(w_vT, "wv")
                for p in range(JT):
                    vt_ps = qkvps.tile([128, C], F32, tag="mm")
                    for c in range(CT):
                        nc.tensor.matmul(vt_ps,
                                         hf[c][:, p * 128:(p + 1) * 128],
                                         wv[c], start=(c == 0), stop=False)
                    nc.tensor.matmul(vt_ps, ones_row, bvr, start=False, stop=True)
                    vt_sb = qkvsb.tile([128, C], MM, tag="vt")
                    nc.scalar.copy(out=vt_sb, in_=vt_ps)
                    nc.sync.dma_start(out=vt_dram[p], in_=vt_sb)

                # k = w_kT^T hf + b_k : resident [c,n] tiles
                wk = load_w(w_kT, "wk")
                for o in range(CT):
                    for p in range(PT):
                        k_ps = qkvps.tile([128, 512], F32, tag="mm")
                        for c in range(CT):
                            nc.tensor.matmul(k_ps,
                                             wk[c][:, o * 128:(o + 1) * 128],
                                             hf[c][:, p * 512:(p + 1) * 512],
                                             start=(c == 0), stop=(c == CT - 1))
                        nc.vector.tensor_scalar_add(
                            out=k_tiles[o][:, p * 512:(p + 1) * 512],
                            in0=k_ps, scalar1=bkcol[o])

                # q = w_qT^T hf + b_q for first NQ columns -> DRAM
                wq = load_w(w_qT, "wq")
                for o in range(CT):
                    for p in range(QT):
                        q_ps = qkvps.tile([128, 512], F32, tag="mm")
                        for c in range(CT):
                            nc.tensor.matmul(q_ps,
                                             wq[c][:, o * 128:(o + 1) * 128],
                                             hf[c][:, p * 512:(p + 1) * 512],
                                             start=(c == 0), stop=(c == CT - 1))
                        q_sb = qkvsb.tile([128, 512], MM, tag="q")
                        nc.vector.tensor_scalar_add(out=q_sb, in0=q_ps,
                                                    scalar1=bqcol[o])
                        nc.sync.dma_start(
                            out=q_dram[o][:, p * 512:(p + 1) * 512], in_=q_sb)

        # ---------------- attention + proj (per 512-wide q-chunk) ----------------
        with tc.tile_pool(name="qa", bufs=2) as qapool, \
             tc.tile_pool(name="estream", bufs=3) as epool, \
             tc.tile_pool(name="vstream", bufs=4) as vpool, \
             tc.tile_pool(name="osb", bufs=2) as opool, \
             tc.tile_pool(name="ysb", bufs=2) as ypool, \
             tc.tile_pool(name="xbst", bufs=3) as xbpool, \
             tc.tile_pool(name="dsb", bufs=2) as dpool, \
             tc.tile_pool(name="psS", bufs=2, space="PSUM") as psS, \
             tc.tile_pool(name="psO", bufs=1, space="PSUM") as psO, \
             tc.tile_pool(name="psD", bufs=2, space="PSUM") as psD:

            for qc in range(QT):
                # stream this q-chunk back: qa[c] = q[c, qc*512 : +512]
                qa = []
                for c in range(CT):
                    qa_c = qapool.tile([128, 512], MM, name=f"qa_{c}",
                                       tag=f"qa{c}")
                    nc.sync.dma_start(out=qa_c,
                                      in_=q_dram[c][:, qc * 512:(qc + 1) * 512])
                    qa.append(qa_c)

                o_ps = [psO.tile([128, 512], F32, name=f"o_ps{co}",
                                 tag=f"o{co}") for co in range(CT)]
                d_ps = psD.tile([1, 512], F32, tag="d")

                def s_exp_v(j):
                    # S^T chunk + exp, and prefetch v^T tile for this j
                    vt_sb = vpool.tile([128, C], MM, tag="vt2")
                    nc.sync.dma_start(out=vt_sb, in_=vt_dram[j])
                    s_ps = psS.tile([128, 512], F32, tag="s")
                    for c in range(CT):
                        nc.tensor.matmul(s_ps,
                                         k_tiles[c][:, j * 128:(j + 1) * 128],
                                         qa[c], start=(c == 0), stop=(c == CT - 1))
                    e_sb = epool.tile([128, 512], MM, tag="e")
                    nc.scalar.activation(out=e_sb, in_=s_ps, func=AF.Exp,
                                         scale=SCALE)
                    return e_sb, vt_sb

                e_cur, v_cur = s_exp_v(0)
                for j in range(JT):
                    nxt = s_exp_v(j + 1) if j + 1 < JT else None
                    first, last = (j == 0), (j == JT - 1)
                    for co in range(CT):
                        nc.tensor.matmul(o_ps[co],
                                         v_cur[:, co * 128:(co + 1) * 128],
                                         e_cur, start=first, stop=last)
                    nc.tensor.matmul(d_ps, ones_col, e_cur,
                                     start=first, stop=last)
                    if nxt is not None:
                        e_cur, v_cur = nxt

                # denominators -> per-query reciprocal [128,1] per 128-row block
                d_sb = dpool.tile([1, 512], F32, tag="dsb")
                nc.scalar.copy(out=d_sb, in_=d_ps)
                rc = []
                for qs in range(4):
                    dt_ps = psS.tile([128, 1], F32, name=f"dt_ps{qs}", tag="s")
                    nc.tensor.transpose(dt_ps,
                                        d_sb[0:1, qs * 128:(qs + 1) * 128],
                                        one1)
                    rc_t = dpool.tile([128, 1], F32, name=f"rc_{qs}",
                                      tag=f"rc{qs}")
                    nc.vector.reciprocal(out=rc_t, in_=dt_ps)
                    rc.append(rc_t)

                # O -> SBUF (rounds to MM dtype)
                o_sb = []
                for co in range(CT):
                    o_t = opool.tile([128, 512], MM, name=f"o_sb{co}",
                                     tag=f"ob{co}")
                    nc.vector.tensor_copy(o_t, o_ps[co])
                    o_sb.append(o_t)

                # proj + 1/denom + residual, per 128-row output block
                for qs in range(4):
                    y_ps = psO.tile([128, C], F32, name=f"y_ps{qs}",
                                    tag=f"o{qs}")
                    for c in range(CT):
                        nc.tensor.matmul(y_ps,
                                         o_sb[c][:, qs * 128:(qs + 1) * 128],
                                         wp[c], start=(c == 0), stop=(c == CT - 1))
                    row0 = qc * 512 + qs * 128
                    xb_sb = xbpool.tile([128, C], F32, tag="xb")
                    nc.sync.dma_start(out=xb_sb, in_=xb_t[row0:row0 + 128, :])
                    y1 = ypool.tile([128, C], F32, tag="y1")
                    nc.vector.tensor_scalar_mul(out=y1, in0=y_ps, scalar1=rc[qs])
                    yo = ypool.tile([128, C], F32, tag="yo")
                    nc.vector.tensor_add(yo, y1, xb_sb)
                    nc.sync.dma_start(out=y_t[row0:row0 + 128, :], in_=yo)

    nc.compile()
    return nc


def _get_prog():
    global _PROG, _PROG_DT
    if _PROG is None or _PROG_DT != MM_DT_NAME:
        _PROG = _build_program(MM_DT_NAME)
        _PROG_DT = MM_DT_NAME
    return _PROG


def _round_f32r(a):
    """RNE to 11 explicit mantissa bits (the fp32r matmul input format)."""
    if MM_DT_NAME != "float32r":
        return a
    b = np.ascontiguousarray(a, dtype=np.float32).view(np.uint32)
    shift = 12
    lsb = (b >> shift) & 1
    mask = np.uint32((~((1 << shift) - 1)) & 0xFFFFFFFF)
    out = (b + np.uint32((1 << (shift - 1)) - 1) + lsb) & mask
    return out.view(np.float32)


def kernel(x, gamma, beta, w_qkv, b_qkv, w_proj, b_proj):
    from concourse.bass_utils import run_bass_kernel_spmd

    x = np.asarray(x, dtype=np.float32)
    gamma = np.asarray(gamma, dtype=np.float32)
    beta = np.asarray(beta, dtype=np.float32)
    w_qkv = np.asarray(w_qkv, dtype=np.float32)
    b_qkv = np.asarray(b_qkv, dtype=np.float32)
    w_proj = np.asarray(w_proj, dtype=np.float32)
    b_proj = np.asarray(b_proj, dtype=np.float32)

    shared = {
        "w_qT": _round_f32r(w_qkv[0:C].T),
        "w_kT": _round_f32r(w_qkv[C:2 * C].T),
        "w_vT": _round_f32r(w_qkv[2 * C:3 * C].T),
        "w_pT": _round_f32r(w_proj.T),
        "b_q": np.ascontiguousarray(b_qkv[0:C].reshape(C, 1)),
        "b_k": np.ascontiguousarray(b_qkv[C:2 * C].reshape(C, 1)),
        "b_v": _round_f32r(b_qkv[2 * C:3 * C].reshape(1, C)),
        "gam": np.ascontiguousarray(gamma.reshape(C, 1)),
        "bet": np.ascontiguousarray(beta.reshape(C, 1)),
        "gmat": (np.arange(128)[:, None] // GSIZE ==
                 np.arange(8)[None, :]).astype(np.float32),
        "gmat_t": np.ascontiguousarray(
            (np.arange(128)[:, None] // GSIZE ==
             np.arange(8)[None, :]).astype(np.float32).T),
    }

    in_maps = []
    for i in range(NCORES):
        b, h = i // 2, i % 2
        x2 = x[b].reshape(C, N)
        if h == 0:
            x_cn = np.ascontiguousarray(x2)
        else:
            x_cn = np.ascontiguousarray(
                np.concatenate([x2[:, NQ:], x2[:, :NQ]], axis=1))
        xb = np.ascontiguousarray(x2.T[h * NQ:(h + 1) * NQ] + b_proj[None, :])
        m = {"x_cn": x_cn, "xb_t": xb}
        m.update(shared)
        in_maps.append(m)

    nc = _get_prog()
    trace = os.environ.get("KERNEL_TRACE", "0") == "1"
    res = run_bass_kernel_spmd(nc, in_maps, list(range(NCORES)), trace=trace)
    if trace:
        kernel.last_exec_time_ns = res.exec_time_ns
        kernel.last_results = res

    out = np.empty((B, C, N), dtype=np.float32)
    for i in range(NCORES):
        b, h = i // 2, i % 2
        out[b][:, h * NQ:(h + 1) * NQ] = res.results[i]["y_t"].T
    return out.reshape(B, C, HH, WW)


# revision 12
# speedup vs baseline: 1.0788x; 1.0776x over previous
"""Trainium2 Bass kernel for nn_AttentionBlock (GroupNorm + single-head self-attention + proj + residual).

Input  x [4, 512, 64, 64] f32.  8 NeuronCores: core i handles batch b=i//2,
query-half h=i%2 (2048 of the 4096 spatial positions).  Each core computes
GroupNorm + full K/V for its batch element, Q only for its half, flash-style
attention over all 4096 keys, the output projection and residual for its half.
No collectives: host shards inputs / gathers outputs.

Layouts (per core), n=4096, nq=2048, c=512:
  hf  = GN(x)      [c, n]    (4 SBUF tiles [128, 4096], in-place over x)
  k   = Wk hf + bk [c, n]    (resident SBUF, 4 tiles [128, 4096])
  q   = Wq hf + bq [c, nq]   (spilled to DRAM, streamed back per q-chunk)
  v^T = (Wv hf + bv)^T [n, c] (spilled to DRAM, streamed back per (qc, j))
  S^T chunk [j:128, q:512] = k_chunk^T q_chunk   (PSUM)
  E = exp(S^T * 1/sqrt(c))  -> O[co,q] += v^T_j[:,co]^T E ;  denom[1,q] += 1^T E
  y^T [q:128, c:512] = (O chunk)^T Wp^T ; y = y^T * (1/denom) + (x^T + b_proj)
"""

import os
import numpy as np

B, C, HH, WW = 4, 512, 64, 64
N = HH * WW            # 4096
NQ = N // 2            # 2048 queries per core
NCORES = 8
CT = C // 128          # 4 channel tiles
PT = N // 512          # 8 spatial chunks of 512
QT = NQ // 512         # 4 query chunks of 512
JT = N // 128          # 32 key chunks of 128
GSIZE = 16             # channels per group
EPS = 1e-5
SCALE = 1.0 / float(np.sqrt(C))

MM_DT_NAME = os.environ.get("KERNEL_MM_DT", "float32")

_PROG = None
_PROG_DT = None


def _build_program(mm_dt_name):
    import concourse.bacc as bacc
    import concourse.tile as tile
    from concourse import mybir
    from contextlib import ExitStack

    F32 = mybir.dt.float32
    MM = getattr(mybir.dt, mm_dt_name)

    nc = bacc.Bacc("TRN2", target_bir_lowering=False, debug=False,
                   num_devices=NCORES)

    def din(name, shape, dt=None):
        return nc.dram_tensor(name, shape, dt or F32, kind="ExternalInput").ap()

    x_cn = din("x_cn", [C, N])          # x for this batch, query-half first
    xb_t = din("xb_t", [NQ, C])         # x^T residual slice + b_proj
    w_qT = din("w_qT", [C, C], MM)
    w_kT = din("w_kT", [C, C], MM)
    w_vT = din("w_vT", [C, C], MM)
    w_pT = din("w_pT", [C, C], MM)
    b_q = din("b_q", [C, 1])
    b_k = din("b_k", [C, 1])
    b_v = din("b_v", [1, C], MM)
    gam = din("gam", [C, 1])
    bet = din("bet", [C, 1])
    gmat = din("gmat", [128, 8])        # group membership (p//16 == u)
    gmat_t = din("gmat_t", [8, 128])
    y_t = nc.dram_tensor("y_t", [NQ, C], F32, kind="ExternalOutput").ap()

    AF = mybir.ActivationFunctionType
    OP = mybir.AluOpType

    with tile.TileContext(nc) as tc, ExitStack() as ctx:
        persist = ctx.enter_context(tc.tile_pool(name="persist", bufs=1))
        kpool = ctx.enter_context(tc.tile_pool(name="kpool", bufs=1))
        dram = ctx.enter_context(tc.tile_pool(name="dram", bufs=1, space="DRAM"))

        # ---- persistent small constants ----
        gma = persist.tile([128, 8], F32)
        nc.sync.dma_start(out=gma, in_=gmat)
        gmt = persist.tile([8, 128], F32)
        nc.sync.dma_start(out=gmt, in_=gmat_t)
        one1 = persist.tile([1, 1], F32)
        nc.vector.memset(one1, 1.0)
        ones_row = persist.tile([1, 128], MM)
        ones_col = persist.tile([128, 1], MM)
        if MM is F32:
            nc.vector.memset(ones_row, 1.0)
            nc.vector.memset(ones_col, 1.0)
        else:
            ones_st_r = persist.tile([1, 128], F32)
            nc.vector.memset(ones_st_r, 1.0)
            nc.vector.tensor_copy(ones_row, ones_st_r)
            ones_st_c = persist.tile([128, 1], F32)
            nc.vector.memset(ones_st_c, 1.0)
            nc.vector.tensor_copy(ones_col, ones_st_c)
        eps8 = persist.tile([8, 1], F32)
        nc.vector.memset(eps8, EPS)
        warm_a = persist.tile([128, 128], mybir.dt.bfloat16)
        nc.vector.memset(warm_a, 0.03)
        gcol = []
        bcol = []
        bqcol = []
        bkcol = []
        for t in range(CT):
            g_t = persist.tile([128, 1], F32, name=f"g_{t}")
            nc.sync.dma_start(out=g_t, in_=gam[t * 128:(t + 1) * 128, :])
            gcol.append(g_t)
            be_t = persist.tile([128, 1], F32, name=f"be_{t}")
            nc.sync.dma_start(out=be_t, in_=bet[t * 128:(t + 1) * 128, :])
            bcol.append(be_t)
            bq_t = persist.tile([128, 1], F32, name=f"bq_{t}")
            nc.sync.dma_start(out=bq_t, in_=b_q[t * 128:(t + 1) * 128, :])
            bqcol.append(bq_t)
            bk_t = persist.tile([128, 1], F32, name=f"bk_{t}")
            nc.sync.dma_start(out=bk_t, in_=b_k[t * 128:(t + 1) * 128, :])
            bkcol.append(bk_t)
        # b_v row (already MM dtype in DRAM)
        bvr = persist.tile([1, C], MM)
        nc.sync.dma_start(out=bvr, in_=b_v)
        # w_proj^T tiles (persist: used at the end of every q-chunk)
        wp = []
        for t in range(CT):
            wp_t = persist.tile([128, C], MM, name=f"wp_{t}")
            nc.sync.dma_start(out=wp_t, in_=w_pT[t * 128:(t + 1) * 128, :])
            wp.append(wp_t)

        # k stays resident for the whole kernel
        k_tiles = [kpool.tile([128, N], MM, name=f"k_{t}", tag=f"k{t}")
                   for t in range(CT)]
        # spill targets
        vt_dram = dram.tile([JT, 128, C], MM)     # v^T as 32 j-tiles [128, 512]
        q_dram = dram.tile([CT, 128, NQ], MM)     # q in [c, nq] layout

        with tc.tile_pool(name="hfpool", bufs=1) as hfpool:
            hf = [hfpool.tile([128, N], MM, name=f"hf_{t}", tag=f"hf{t}")
                  for t in range(CT)]

            # ---------------- GroupNorm ----------------
            # per c-tile: load x, per-channel stats, group-aggregate (groups
            # never span c-tiles), expand, apply -> hf (rounded to MM dtype)
            with tc.tile_pool(name="xpool", bufs=2) as xpool, \
                 tc.tile_pool(name="gnsb", bufs=2) as gnsb, \
                 tc.tile_pool(name="gnps", bufs=2, space="PSUM") as gnps:
                for t in range(CT):
                    x_t = xpool.tile([128, N], F32, tag="x")
                    nc.sync.dma_start(out=x_t,
                                      in_=x_cn[t * 128:(t + 1) * 128, :])
                    stats = gnsb.tile([128, PT, 6], F32, tag="stats")
                    for s in range(PT):
                        nc.vector.bn_stats(out=stats[:, s, :],
                                           in_=x_t[:, s * 512:(s + 1) * 512])
                    mv = gnsb.tile([128, 2], F32, tag="mv")
                    nc.vector.bn_aggr(out=mv, in_=stats)
                    # st2 = [mean, E[x^2]] per channel
                    st2 = gnsb.tile([128, 2], F32, tag="st2")
                    nc.vector.tensor_copy(st2[:, 0:1], mv[:, 0:1])
                    msq = gnsb.tile([128, 1], F32, tag="msq")
                    nc.vector.tensor_mul(msq, mv[:, 0:1], mv[:, 0:1])
                    nc.vector.tensor_add(st2[:, 1:2], mv[:, 1:2], msq)
                    # group-aggregate: [8, 2] sums over the 16 channels of each group
                    gps = gnps.tile([8, 2], F32, tag="gps")
                    nc.tensor.matmul(gps, gma, st2, start=True, stop=True)
                    grp = gnsb.tile([8, 2], F32, tag="grp")
                    nc.scalar.mul(out=grp, in_=gps, mul=1.0 / GSIZE)
                    gm2 = gnsb.tile([8, 1], F32, tag="gm2")
                    nc.vector.tensor_mul(gm2, grp[:, 0:1], grp[:, 0:1])
                    var = gnsb.tile([8, 1], F32, tag="var")
                    nc.vector.tensor_sub(var, grp[:, 1:2], gm2)
                    std = gnsb.tile([8, 1], F32, tag="std")
                    nc.scalar.activation(out=std, in_=var, func=AF.Sqrt,
                                         bias=eps8, scale=1.0)
                    gout = gnsb.tile([8, 2], F32, tag="gout")
                    nc.vector.tensor_copy(gout[:, 0:1], grp[:, 0:1])
                    nc.vector.reciprocal(out=gout[:, 1:2], in_=std)
                    # expand group stats back to per-channel [128, 2]
                    eps_ps = gnps.tile([128, 2], F32, tag="eps_ps")
                    nc.tensor.matmul(eps_ps, gmt, gout, start=True, stop=True)
                    pg = gnsb.tile([128, 2], F32, tag="pg")
                    nc.scalar.copy(out=pg, in_=eps_ps)
                    # per-channel scale = gamma*rstd ; bias = beta - mean*scale
                    sc = gnsb.tile([128, 1], F32, tag="sc")
                    nc.vector.tensor_mul(sc, gcol[t], pg[:, 1:2])
                    bc = gnsb.tile([128, 1], F32, tag="bc")
                    nc.vector.tensor_mul(bc, pg[:, 0:1], sc)
                    nc.vector.tensor_sub(bc, bcol[t], bc)
                    # apply: hf = x*sc + bc  (rounds to MM dtype)
                    nc.vector.tensor_scalar(out=hf[t], in0=x_t,
                                            scalar1=sc, scalar2=bc,
                                            op0=OP.mult, op1=OP.add)
                    # HAM warm-up: dense bf16 matmuls chained to hf[t] keep
                    # the PE activity monitor in the fast-clock state through
                    # the (otherwise PE-idle) GroupNorm phase.
                    wb = gnsb.tile([128, 512], mybir.dt.bfloat16, tag="warmb")
                    nc.vector.tensor_copy(wb, hf[t][:, 0:512])
                    wps = gnps.tile([128, 512], F32, tag="warmps")
                    for wi in range(10):
                        nc.tensor.matmul(wps, warm_a, wb,
                                         start=(wi == 0), stop=(wi == 9))

            # ---------------- QKV ----------------
            with tc.tile_pool(name="wmat", bufs=1) as wmat, \
                 tc.tile_pool(name="qkvsb", bufs=3) as qkvsb, \
                 tc.tile_pool(name="qkvps", bufs=4, space="PSUM") as qkvps:

                def load_w(src, nm):
                    tiles = []
                    for t in range(CT):
                        w_t = wmat.tile([128, C], MM, name=f"{nm}{t}",
                                        tag=f"w{t}", bufs=2)
                        nc.sync.dma_start(out=w_t,
                                          in_=src[t * 128:(t + 1) * 128, :])
                        tiles.append(w_t)
                    return tiles

                # v^T = hf^T w_vT + b_v : 32 tiles [128p, 512c] -> DRAM
                wv = load_w(w_vT, "wv")
                for p in range(JT):
                    vt_ps = qkvps.tile([128, C], F32, tag="mm")
                    for c in range(CT):
                        nc.tensor.matmul(vt_ps,
                                         hf[c][:, p * 128:(p + 1) * 128],
                                         wv[c], start=(c == 0), stop=False)
                    nc.tensor.matmul(vt_ps, ones_row, bvr, start=False, stop=True)
                    vt_sb = qkvsb.tile([128, C], MM, tag="vt")
                    nc.scalar.copy(out=vt_sb, in_=vt_ps)
                    nc.sync.dma_start(out=vt_dram[p], in_=vt_sb)

                # k = w_kT^T hf + b_k : resident [c,n] tiles
                wk = load_w(w_kT, "wk")
                for o in range(CT):
                    for p in range(PT):
                        k_ps = qkvps.tile([128, 512], F32, tag="mm")
                        for c in range(CT):
                            nc.tensor.matmul(k_ps,
                                             wk[c][:, o * 128:(o + 1) * 128],
                                             hf[c][:, p * 512:(p + 1) * 512],
                                             start=(c == 0), stop=(c == CT - 1))
                        nc.vector.tensor_scalar_add(
                            out=k_tiles[o][:, p * 512:(p + 1) * 512],
                            in0=k_ps, scalar1=bkcol[o])

                # q = w_qT^T hf + b_q for first NQ columns -> DRAM
                wq = load_w(w_qT, "wq")
                for o in range(CT):
                    for p in range(QT):
                        q_ps = qkvps.tile([128, 512], F32, tag="mm")
                        for c in range(CT):
                            nc.tensor.matmul(q_ps,
                                             wq[c][:, o * 128:(o + 1) * 128],
                                             hf[c][:, p * 512:(p + 1) * 512],
                                             start=(c == 0), stop=(c == CT - 1))
                        q_sb = qkvsb.tile([128, 512], MM, tag="q")
                        nc.vector.tensor_scalar_add(out=q_sb, in0=q_ps,
                                                    scalar1=bqcol[o])
                        nc.sync.dma_start(
                            out=q_dram[o][:, p * 512:(p + 1) * 512], in_=q_sb)

        # ---------------- attention + proj (per 512-wide q-chunk) ----------------
        with tc.tile_pool(name="qa", bufs=2) as qapool, \
             tc.tile_pool(name="estream", bufs=3) as epool, \
             tc.tile_pool(name="vstream", bufs=4) as vpool, \
             tc.tile_pool(name="osb", bufs=2) as opool, \
             tc.tile_pool(name="ysb", bufs=2) as ypool, \
             tc.tile_pool(name="xbst", bufs=3) as xbpool, \
             tc.tile_pool(name="dsb", bufs=2) as dpool, \
             tc.tile_pool(name="psS", bufs=2, space="PSUM") as psS, \
             tc.tile_pool(name="psO", bufs=1, space="PSUM") as psO, \
             tc.tile_pool(name="psD", bufs=1, space="PSUM") as psD, \
             tc.tile_pool(name="psY", bufs=1, space="PSUM") as psY:

            for qc in range(QT):
                # stream this q-chunk back: qa[c] = q[c, qc*512 : +512]
                qa = []
                for c in range(CT):
                    qa_c = qapool.tile([128, 512], MM, name=f"qa_{c}",
                                       tag=f"qa{c}")
                    nc.sync.dma_start(out=qa_c,
                                      in_=q_dram[c][:, qc * 512:(qc + 1) * 512])
                    qa.append(qa_c)

                o_ps = [psO.tile([128, 512], F32, name=f"o_ps{co}",
                                 tag=f"o{co}") for co in range(CT)]
                d_ps = psD.tile([1, 512], F32, tag="d")

                def s_exp_v(j):
                    # S^T chunk + exp, and prefetch v^T tile for this j
                    vt_sb = vpool.tile([128, C], MM, tag="vt2")
                    nc.sync.dma_start(out=vt_sb, in_=vt_dram[j])
                    s_ps = psS.tile([128, 512], F32, tag="s")
                    for c in range(CT):
                        nc.tensor.matmul(s_ps,
                                         k_tiles[c][:, j * 128:(j + 1) * 128],
                                         qa[c], start=(c == 0), stop=(c == CT - 1))
                    e_sb = epool.tile([128, 512], MM, tag="e")
                    nc.scalar.activation(out=e_sb, in_=s_ps, func=AF.Exp,
                                         scale=SCALE)
                    return e_sb, vt_sb

                e_cur, v_cur = s_exp_v(0)
                for j in range(JT):
                    nxt = s_exp_v(j + 1) if j + 1 < JT else None
                    first, last = (j == 0), (j == JT - 1)
                    for co in range(CT):
                        nc.tensor.matmul(o_ps[co],
                                         v_cur[:, co * 128:(co + 1) * 128],
                                         e_cur, start=first, stop=last)
                    nc.tensor.matmul(d_ps, ones_col, e_cur,
                                     start=first, stop=last)
                    if nxt is not None:
                        e_cur, v_cur = nxt

                # denominators -> per-query reciprocal [128,1] per 128-row block
                d_sb = dpool.tile([1, 512], F32, tag="dsb")
                nc.scalar.copy(out=d_sb, in_=d_ps)
                rc = []
                for qs in range(4):
                    dt_ps = psD.tile([128, 1], F32, name=f"dt_ps{qs}", tag="d")
                    nc.tensor.transpose(dt_ps,
                                        d_sb[0:1, qs * 128:(qs + 1) * 128],
                                        one1)
                    rc_t = dpool.tile([128, 1], F32, name=f"rc_{qs}",
                                      tag=f"rc{qs}")
                    nc.vector.reciprocal(out=rc_t, in_=dt_ps)
                    rc.append(rc_t)

                # O -> SBUF (rounds to MM dtype)
                o_sb = []
                for co in range(CT):
                    o_t = opool.tile([128, 512], MM, name=f"o_sb{co}",
                                     tag=f"ob{co}")
                    nc.vector.tensor_copy(o_t, o_ps[co])
                    o_sb.append(o_t)

                # proj + 1/denom + residual, per 128-row output block
                for qs in range(4):
                    y_ps = psY.tile([128, C], F32, name=f"y_ps{qs}",
                                    tag="y")
                    for c in range(CT):
                        nc.tensor.matmul(y_ps,
                                         o_sb[c][:, qs * 128:(qs + 1) * 128],
                                         wp[c], start=(c == 0), stop=(c == CT - 1))
                    row0 = qc * 512 + qs * 128
                    xb_sb = xbpool.tile([128, C], F32, tag="xb")
                    nc.sync.dma_start(out=xb_sb, in_=xb_t[row0:row0 + 128, :])
                    y1 = ypool.tile([128, C], F32, tag="y1")
                    nc.vector.tensor_scalar_mul(out=y1, in0=y_ps, scalar1=rc[qs])
                    yo = ypool.tile([128, C], F32, tag="yo")
                    nc.vector.tensor_add(yo, y1, xb_sb)
                    nc.sync.dma_start(out=y_t[row0:row0 + 128, :], in_=yo)

    nc.compile()
    return nc


def _get_prog():
    global _PROG, _PROG_DT
    if _PROG is None or _PROG_DT != MM_DT_NAME:
        _PROG = _build_program(MM_DT_NAME)
        _PROG_DT = MM_DT_NAME
    return _PROG


def _round_f32r(a):
    """RNE to 11 explicit mantissa bits (the fp32r matmul input format)."""
    if MM_DT_NAME != "float32r":
        return a
    b = np.ascontiguousarray(a, dtype=np.float32).view(np.uint32)
    shift = 12
    lsb = (b >> shift) & 1
    mask = np.uint32((~((1 << shift) - 1)) & 0xFFFFFFFF)
    out = (b + np.uint32((1 << (shift - 1)) - 1) + lsb) & mask
    return out.view(np.float32)


def kernel(x, gamma, beta, w_qkv, b_qkv, w_proj, b_proj):
    from concourse.bass_utils import run_bass_kernel_spmd

    x = np.asarray(x, dtype=np.float32)
    gamma = np.asarray(gamma, dtype=np.float32)
    beta = np.asarray(beta, dtype=np.float32)
    w_qkv = np.asarray(w_qkv, dtype=np.float32)
    b_qkv = np.asarray(b_qkv, dtype=np.float32)
    w_proj = np.asarray(w_proj, dtype=np.float32)
    b_proj = np.asarray(b_proj, dtype=np.float32)

    shared = {
        "w_qT": _round_f32r(w_qkv[0:C].T),
        "w_kT": _round_f32r(w_qkv[C:2 * C].T),
        "w_vT": _round_f32r(w_qkv[2 * C:3 * C].T),
        "w_pT": _round_f32r(w_proj.T),
        "b_q": np.ascontiguousarray(b_qkv[0:C].reshape(C, 1)),
        "b_k": np.ascontiguousarray(b_qkv[C:2 * C].reshape(C, 1)),
        "b_v": _round_f32r(b_qkv[2 * C:3 * C].reshape(1, C)),
        "gam": np.ascontiguousarray(gamma.reshape(C, 1)),
        "bet": np.ascontiguousarray(beta.reshape(C, 1)),
        "gmat": (np.arange(128)[:, None] // GSIZE ==
                 np.arange(8)[None, :]).astype(np.float32),
        "gmat_t": np.ascontiguousarray(
            (np.arange(128)[:, None] // GSIZE ==
             np.arange(8)[None, :]).astype(np.float32).T),
    }

    in_maps = []
    for i in range(NCORES):
        b, h = i // 2, i % 2
        x2 = x[b].reshape(C, N)
        if h == 0:
            x_cn = np.ascontiguousarray(x2)
        else:
            x_cn = np.ascontiguousarray(
                np.concatenate([x2[:, NQ:], x2[:, :NQ]], axis=1))
        xb = np.ascontiguousarray(x2.T[h * NQ:(h + 1) * NQ] + b_proj[None, :])
        m = {"x_cn": x_cn, "xb_t": xb}
        m.update(shared)
        in_maps.append(m)

    nc = _get_prog()
    trace = os.environ.get("KERNEL_TRACE", "0") == "1"
    res = run_bass_kernel_spmd(nc, in_maps, list(range(NCORES)), trace=trace)
    if trace:
        kernel.last_exec_time_ns = res.exec_time_ns
        kernel.last_results = res

    out = np.empty((B, C, N), dtype=np.float32)
    for i in range(NCORES):
        b, h = i // 2, i % 2
        out[b][:, h * NQ:(h + 1) * NQ] = res.results[i]["y_t"].T
    return out.reshape(B, C, HH, WW)
